# revision 10
# baseline (speedup 1.0000x reference)
"""CRF loss kernel for Trainium2 (8 NeuronCores, data-parallel over batch).

Reference computation (see problem):
    score = einsum('blf,fk->blk', X, W);  forward/backward CRF messages over L;
    loss = mean_b(emit + trans - logZ).

The per-exec harness cost is dominated by per-tensor staging overhead, so ALL
inputs ship as ONE int8 tensor per core (~4.2 MiB):
  cols [0, 32768)      X codes: round(X / XSCALE), X^T in
                       [F=128, (tile, group, t, b)] layout
  cols [32768, 33024)  YT labels y[b,t] packed [4,8192]->[128,256]
  cols [33024, 33572)  A-plane of bf16 consts CB (round(c))
  cols [33572, 34120)  B-plane (round((c-A)*128)); device cb = A + B/128
CB = [W pad | blockdiag(T) | blockdiag(expT/s) | group-sum | group-select E |
label iota]; s rescales expT into int8-decomp range and is corrected on host.

Device algorithm (per core, batch shard of 1024):
  - decode consts + label rows (DVE), stream X tiles and decode int8->bf16
    on ACT with scale.
  - 16 pipelined tiles of 512 columns (2 timesteps x 256 batch, 4 batch
    groups packed on partitions at offsets 0/32/64/96):
      score psum = W^T @ XT tile (4 matmuls, tile_position packing)
      expsc = exp(score - SHIFT) via ACT (fused PSUM->SBUF), bf16
      masks on device: yrep = E^T @ YT row (partition-broadcast), then
      oh = is_equal(yrep, iota) on DVE into a per-timestep buffer whose
      one-block-shifted slice doubles as the prev-label mask
      tcol = blockdiag(T)^T @ ohp accumulated INTO the score psum after the
      exp read; gold tile sum = reduce((score + tcol) .* oh)
  - CRF forward recursion in probability domain, interleaved with the tile
    loop: p_t = (BD^T @ p_{t-1}) * expsc_t, renormalized every 3 steps by
    group-sum Z, accumulating log Z via ACT Ln accum_out.
  - out[2,1]: [32*sum_b sum log Z, emit+trans total]
Host: loss = sum_cores(gold - sumlog/32 - BC*L*SHIFT - BC*(L-1)*ln s) / B.
"""

import numpy as np

B, L, F, K = 8192, 32, 128, 26
N_CORES = 8
BC = B // N_CORES            # batch per core
GROUPS = 4                   # batch groups packed on partition blocks
GB = BC // GROUPS            # 256 batch columns per group
NT = L // 2                  # 16 tiles, 2 timesteps each
TILE_COLS = 2 * GB           # 512 columns per tile
SHIFT = 26.0
XCLIP = 5.7
XSCALE = XCLIP / 127.0

XCOL = NT * TILE_COLS * GROUPS          # 32768 X-code columns
MCOL = NT * TILE_COLS                   # 8192 mask/expsc columns
YW = MCOL * GROUPS // 128               # 256 packed label columns
CBW = 548                               # const columns
C_YT = XCOL
C_CBA = C_YT + YW
C_CBB = C_CBA + CBW
WT = C_CBB + CBW                        # 34120 total columns

_cache = {}


def _build_program():
    import concourse.bass as bass  # noqa: F401
    import concourse.bacc as bacc
    import concourse.tile as tile
    from concourse import mybir
    from contextlib import ExitStack

    f32 = mybir.dt.float32
    bf16 = mybir.dt.bfloat16
    i8 = mybir.dt.int8
    AF = mybir.ActivationFunctionType
    ALU = mybir.AluOpType
    X_AX = mybir.AxisListType.X

    nc = bacc.Bacc("TRN2", target_bir_lowering=False)

    ALLd = nc.dram_tensor("ALL", [128, WT], i8, kind="ExternalInput")
    OUTd = nc.dram_tensor("out", [2, 1], f32, kind="ExternalOutput")

    with tile.TileContext(nc) as tc, ExitStack() as ctx:
        singles = ctx.enter_context(tc.tile_pool(name="singles", bufs=1))

        # ---- const + label decode ----
        cba8 = singles.tile([128, CBW], i8)
        nc.sync.dma_start(out=cba8, in_=ALLd.ap()[:, C_CBA:C_CBB])
        cbb8 = singles.tile([128, CBW], i8)
        nc.sync.dma_start(out=cbb8, in_=ALLd.ap()[:, C_CBB:C_CBB + CBW])
        cbtmp = singles.tile([128, CBW], bf16)
        nc.vector.tensor_scalar(cbtmp, cbb8, scalar1=1.0 / 128.0, scalar2=None,
                                op0=ALU.mult)
        cba = singles.tile([128, CBW], bf16)
        nc.vector.tensor_copy(out=cba, in_=cba8)
        cb = singles.tile([128, CBW], bf16)
        nc.vector.tensor_tensor(cb, cba, cbtmp, ALU.add)

        wblk = cb[:, 0:32]
        tb = cb[:, 32:160]
        bd = cb[:, 160:288]
        zs = cb[:, 288:416]
        egrp = cb[0:GROUPS, 416:544]        # E[g, 32g'+k] = (g == g')
        iota = cb[:, 544:545]               # partition index % 32

        yt8 = singles.tile([GROUPS, MCOL], i8)
        nc.scalar.dma_start(out=yt8, in_=ALLd.ap()[:, C_YT:C_YT + YW])
        yt = singles.tile([GROUPS, MCOL], bf16)
        nc.vector.tensor_copy(out=yt, in_=yt8)
        # one-hot per timestep: block j holds onehot(y at t=j-1); block 0 = 0
        ohbuf = singles.tile([128, (L + 1) * GB], bf16)
        nc.vector.memset(ohbuf[:, 0:GB], 0.0)

        expsc = singles.tile([128, MCOL], bf16)

        nshift = singles.tile([128, 1], f32)
        nc.vector.memset(nshift, -SHIFT)
        ones = singles.tile([128, 1], f32)
        nc.vector.memset(ones, 1.0)
        goldacc = singles.tile([128, NT], f32)
        logacc = singles.tile([128, 16], f32)
        nc.vector.memset(logacc, 0.0)
        combo = singles.tile([128, 2], f32)

        with tc.tile_pool(name="x8", bufs=3) as x8p, \
             tc.tile_pool(name="xt", bufs=3) as xtp, \
             tc.tile_pool(name="scp", bufs=2, space="PSUM") as scp, \
             tc.tile_pool(name="yrp", bufs=2, space="PSUM") as yrp, \
             tc.tile_pool(name="mp", bufs=2) as mp, \
             tc.tile_pool(name="up", bufs=2, space="PSUM") as up, \
             tc.tile_pool(name="vp", bufs=2) as vp, \
             tc.tile_pool(name="rzp", bufs=2) as rzp, \
             tc.tile_pool(name="lnp", bufs=2) as lnp, \
             tc.tile_pool(name="pp", bufs=2) as pp:

            p_prev = None
            nidx = 0

            def recursion_step(t):
                nonlocal p_prev, nidx
                u = up.tile([128, 2 * GB], f32, tag="u")
                nc.tensor.matmul(u[:, 0:GB], lhsT=bd, rhs=p_prev,
                                 start=True, stop=True)
                i = t // 2
                e_sl = expsc[:, i * TILE_COLS + (t % 2) * GB:
                             i * TILE_COLS + (t % 2) * GB + GB]
                if t % 3 == 0:
                    v = vp.tile([128, GB], bf16)
                    nc.vector.tensor_mul(v, u[:, 0:GB], e_sl)
                    z = u[:, GB:2 * GB]
                    nc.tensor.matmul(z, lhsT=zs, rhs=v, start=True, stop=True,
                                     skip_group_check=True)
                    rz = rzp.tile([128, GB], f32)
                    nc.vector.reciprocal(rz, z)
                    lnscr = lnp.tile([128, GB], bf16)
                    nc.scalar.activation(lnscr, z, AF.Ln,
                                         accum_out=logacc[:, nidx:nidx + 1])
                    nidx += 1
                    pn = pp.tile([128, GB], bf16)
                    nc.vector.tensor_mul(pn, v, rz)
                else:
                    pn = pp.tile([128, GB], bf16)
                    nc.vector.tensor_mul(pn, u[:, 0:GB], e_sl)
                p_prev = pn

            for i in range(NT):
                cs = slice(i * TILE_COLS, (i + 1) * TILE_COLS)

                x8i = x8p.tile([128, GROUPS * TILE_COLS], i8)
                q = nc.sync if i % 2 == 0 else nc.scalar
                q.dma_start(
                    out=x8i,
                    in_=ALLd.ap()[:, i * GROUPS * TILE_COLS:
                                  (i + 1) * GROUPS * TILE_COLS])
                xti = xtp.tile([128, GROUPS * TILE_COLS], bf16)
                nc.scalar.activation(xti[:, 0:1664], x8i[:, 0:1664],
                                     AF.Copy, scale=float(XSCALE))
                nc.vector.tensor_scalar(xti[:, 1664:2048], x8i[:, 1664:2048],
                                        scalar1=float(XSCALE), scalar2=None,
                                        op0=ALU.mult)

                # one-hot masks: write blocks for t=2i, 2i+1; the prev-label
                # mask is the same buffer shifted one timestep back
                yrep = yrp.tile([128, TILE_COLS], f32, tag="yr")
                nc.tensor.matmul(yrep, lhsT=egrp, rhs=yt[:, cs],
                                 start=True, stop=True)
                nc.vector.tensor_tensor(
                    ohbuf[:, (2 * i + 1) * GB:(2 * i + 3) * GB], yrep,
                    iota.to_broadcast([128, TILE_COLS]), ALU.is_equal)
                oh_t = ohbuf[:, (2 * i + 1) * GB:(2 * i + 3) * GB]
                ohp_t = ohbuf[:, (2 * i) * GB:(2 * i + 2) * GB]

                sc = scp.tile([128, TILE_COLS], f32)
                for g in range(GROUPS):
                    nc.tensor.matmul(
                        sc[32 * g:32 * g + 32, :],
                        lhsT=wblk,
                        rhs=xti[:, g * TILE_COLS:(g + 1) * TILE_COLS],
                        start=True, stop=True,
                        tile_position=(0, 32 * g),
                    )
                nc.scalar.activation(expsc[:, cs], sc, AF.Exp,
                                     bias=nshift[:, 0:1])

                # accumulate T[y_prev, k] into the score psum (after exp read)
                nc.tensor.matmul(sc, lhsT=tb, rhs=ohp_t,
                                 start=False, stop=True,
                                 skip_group_check=True)

                m1 = mp.tile([128, TILE_COLS], f32)
                nc.vector.tensor_tensor(m1, sc, oh_t, ALU.mult)
                nc.vector.tensor_reduce(goldacc[:, i:i + 1], m1,
                                        axis=X_AX, op=ALU.add)

                # recursion steps enabled by this tile
                if i == 0:
                    p_prev = expsc[:, 0:GB]
                    recursion_step(1)
                else:
                    recursion_step(2 * i)
                    recursion_step(2 * i + 1)

            # final partition-function sum (t = L-1 state)
            zf = up.tile([128, 2 * GB], f32, tag="u")
            nc.tensor.matmul(zf[:, 0:GB], lhsT=zs, rhs=p_prev,
                             start=True, stop=True)
            lnscr = lnp.tile([128, GB], bf16)
            nc.scalar.activation(lnscr, zf[:, 0:GB], AF.Ln,
                                 accum_out=logacc[:, nidx:nidx + 1])
            nidx += 1
            assert nidx == 11

            nc.vector.tensor_reduce(combo[:, 0:1], logacc, axis=X_AX,
                                    op=ALU.add)
            nc.vector.tensor_reduce(combo[:, 1:2], goldacc, axis=X_AX,
                                    op=ALU.add)
            res_ps = up.tile([128, 2 * GB], f32, tag="u")
            nc.tensor.matmul(res_ps[0:2, 0:1], lhsT=combo, rhs=ones,
                             start=True, stop=True)
            outsb = singles.tile([2, 1], f32)
            nc.vector.tensor_copy(out=outsb, in_=res_ps[0:2, 0:1])
            nc.sync.dma_start(out=OUTd.ap(), in_=outsb)

    nc.compile()
    return nc


def _get_program():
    if "nc" not in _cache:
        _cache["nc"] = _build_program()
    return _cache["nc"]


def _pack_labels(lab):
    """[4, 8192] int8 label rows -> [128, 256] packed so that a DMA from the
    [128, 256] DRAM slice into a [4, 8192] SBUF tile reproduces them."""
    flat = lab.reshape(-1)                       # j = p*8192 + c
    return flat.reshape(128, YW)                 # row p' holds j = p'*256 ...


def _make_consts(W, T, bd_scale):
    import ml_dtypes
    bf = ml_dtypes.bfloat16
    cb = np.zeros((128, CBW), dtype=np.float64)
    cb[:, :K] = W.astype(bf).astype(np.float64)
    expTs = np.exp(T.astype(np.float64)) / bd_scale
    Tb = T.astype(bf).astype(np.float64)
    for g in range(GROUPS):
        r = slice(32 * g, 32 * g + K)
        cb[r, 32 + 32 * g:32 + 32 * g + K] = Tb       # tb (blockdiag T)
        cb[r, 160 + 32 * g:160 + 32 * g + K] = expTs  # bd (blockdiag expT/s)
        cb[g, 416 + 32 * g:416 + 32 * g + 32] = 1     # egrp (group select)
    for r in range(128):
        g = r // 32
        if r % 32 < K:
            cb[r, 288 + 32 * g:288 + 32 * g + 32] = 1  # zs (group-sum)
        cb[r, 544] = r % 32                            # iota (label index)
    # two-plane int8 decomposition: cb ~= A + B/128
    A = np.clip(np.round(cb), -127, 127)
    Bp = np.clip(np.round((cb - A) * 128.0), -127, 127)
    return A.astype(np.int8), Bp.astype(np.int8)


def _make_in_maps(X, y, W, T):
    W = np.asarray(W)
    T = np.asarray(T)
    bd_scale = max(1.0, float(np.exp(T.astype(np.float64)).max()) / 120.0)
    _cache["bd_scale"] = bd_scale
    cba, cbb = _make_consts(W, T, bd_scale)

    X = np.asarray(X, dtype=np.float32)
    y = np.asarray(y)
    in_maps = []
    for cidx in range(N_CORES):
        Xc = X[cidx * BC:(cidx + 1) * BC]               # [1024, 32, 128]
        Xg = Xc.reshape(GROUPS, GB, L, F)               # [g, b, t, f]
        # X cols = (tile, group, t_local, b): i*2048 + g*512 + tl*256 + b
        XT = (Xg.transpose(3, 2, 0, 1)                  # [f, t, g, b]
                .reshape(F, NT, 2, GROUPS, GB)          # [f, i, tl, g, b]
                .transpose(0, 1, 3, 2, 4)               # [f, i, g, tl, b]
                .reshape(F, XCOL))
        Xq = np.clip(np.round(XT / XSCALE), -127, 127).astype(np.int8)

        Yc = y[cidx * BC:(cidx + 1) * BC].astype(np.int64)  # [1024, 32]
        Yg = Yc.reshape(GROUPS, GB, L)                  # [g, b, t]

        # label rows: [g, (tile, t_local, b)] = [4, i*512 + tl*256 + b]
        def lrows(lbl):
            return (lbl.transpose(0, 2, 1)              # [g, t, b]
                       .reshape(GROUPS, MCOL).astype(np.int8))

        allt = np.empty((128, WT), dtype=np.int8)
        allt[:, :XCOL] = Xq
        allt[:, C_YT:C_YT + YW] = _pack_labels(lrows(Yg))
        allt[:, C_CBA:C_CBA + CBW] = cba
        allt[:, C_CBB:C_CBB + CBW] = cbb
        in_maps.append({"ALL": allt})
    return in_maps


def _combine(results):
    bd_scale = _cache.get("bd_scale", 1.0)
    lncorr = BC * (L - 1) * np.log(bd_scale)
    total = 0.0
    for r in results:
        o = np.asarray(r["out"], dtype=np.float64).reshape(-1)
        sumlog = o[0] / 32.0
        gold = o[1]
        total += gold - (sumlog + lncorr) - BC * L * SHIFT
    return np.float32(total / B)


def kernel(X, y, W, T):
    from concourse.bass_utils import run_bass_kernel_spmd
    nc = _get_program()
    in_maps = _make_in_maps(np.asarray(X), np.asarray(y),
                            np.asarray(W), np.asarray(T))
    res = run_bass_kernel_spmd(nc, in_maps, list(range(N_CORES)))
    return _combine(res.results)


# revision 14
# speedup vs baseline: 1.2332x; 1.2332x over previous
"""CRF loss kernel for Trainium2 (8 NeuronCores, data-parallel over batch).

Reference computation (see problem):
    score = einsum('blf,fk->blk', X, W);  forward/backward CRF messages over L;
    loss = mean_b(emit + trans - logZ).

The per-exec harness cost is dominated by per-tensor staging overhead, so ALL
inputs ship as ONE int8 tensor per core (~4.2 MiB):
  cols [0, 32768)      X codes: round(X / XSCALE), X^T in
                       [F=128, (tile, group, t, b)] layout
  cols [32768, 33792)  YT labels y[b,t] packed [4,8192]->[32,1024] (1KB lines)
  cols [33792, 34340)  A-plane of bf16 consts CB (round(c))
  cols [34340, 34888)  B-plane (round((c-A)*128)); device cb = A + B/128
CB = [W pad | blockdiag(T) | blockdiag(expT/s) | group-sum | group-select E |
label iota]; s rescales expT into int8-decomp range and is corrected on host.

Device algorithm (per core, batch shard of 1024):
  - decode consts + label rows (DVE), stream X tiles and decode int8->bf16
    on ACT with scale.
  - 16 pipelined tiles of 512 columns (2 timesteps x 256 batch, 4 batch
    groups packed on partitions at offsets 0/32/64/96):
      score psum = W^T @ XT tile (4 matmuls, tile_position packing)
      expsc = exp(score - SHIFT) via ACT (fused PSUM->SBUF), bf16
      masks on device: yrep = E^T @ YT row (partition-broadcast), then
      oh = is_equal(yrep, iota) on DVE into a per-timestep buffer whose
      one-block-shifted slice doubles as the prev-label mask
      tcol = blockdiag(T)^T @ ohp accumulated INTO the score psum after the
      exp read; gold tile sum = reduce((score + tcol) .* oh)
  - CRF forward recursion in probability domain, interleaved with the tile
    loop: p_t = (BD^T @ p_{t-1}) * expsc_t, renormalized every 3 steps by
    group-sum Z, accumulating log Z via ACT Ln accum_out.
  - out[2,1]: [32*sum_b sum log Z, emit+trans total]
Host: loss = sum_cores(gold - sumlog/32 - BC*L*SHIFT - BC*(L-1)*ln s) / B.
"""

import numpy as np

B, L, F, K = 8192, 32, 128, 26
N_CORES = 8
BC = B // N_CORES            # batch per core
GROUPS = 4                   # batch groups packed on partition blocks
GB = BC // GROUPS            # 256 batch columns per group
NT = L // 2                  # 16 tiles, 2 timesteps each
TILE_COLS = 2 * GB           # 512 columns per tile
SHIFT = 26.0
XCLIP = 5.7
XSCALE = XCLIP / 127.0

XCOL = NT * TILE_COLS * GROUPS          # 32768 X-code columns
MCOL = NT * TILE_COLS                   # 8192 mask/expsc columns
YW = 1024                              # label region cols (rows 0-31 used)
CBW = 548                               # const columns
C_YT = XCOL
C_CBA = C_YT + YW
C_CBB = C_CBA + CBW
WT = C_CBB + CBW                        # 34120 total columns

_cache = {}


def _build_program():
    import concourse.bass as bass  # noqa: F401
    import concourse.bacc as bacc
    import concourse.tile as tile
    from concourse import mybir
    from contextlib import ExitStack

    f32 = mybir.dt.float32
    bf16 = mybir.dt.bfloat16
    i8 = mybir.dt.int8
    AF = mybir.ActivationFunctionType
    ALU = mybir.AluOpType
    X_AX = mybir.AxisListType.X

    nc = bacc.Bacc("TRN2", target_bir_lowering=False)

    ALLd = nc.dram_tensor("ALL", [128, WT], i8, kind="ExternalInput")
    OUTd = nc.dram_tensor("out", [2, 1], f32, kind="ExternalOutput")

    with tile.TileContext(nc) as tc, ExitStack() as ctx:
        singles = ctx.enter_context(tc.tile_pool(name="singles", bufs=1))

        # ---- const + label decode ----
        cba8 = singles.tile([128, CBW], i8)
        nc.sync.dma_start(out=cba8, in_=ALLd.ap()[:, C_CBA:C_CBB])
        cbb8 = singles.tile([128, CBW], i8)
        nc.sync.dma_start(out=cbb8, in_=ALLd.ap()[:, C_CBB:C_CBB + CBW])
        cbtmp = singles.tile([128, CBW], bf16)
        nc.vector.tensor_scalar(cbtmp, cbb8, scalar1=1.0 / 128.0, scalar2=None,
                                op0=ALU.mult)
        cba = singles.tile([128, CBW], bf16)
        nc.vector.tensor_copy(out=cba, in_=cba8)
        cb = singles.tile([128, CBW], bf16)
        nc.vector.tensor_tensor(cb, cba, cbtmp, ALU.add)

        wblk = cb[:, 0:32]
        tb = cb[:, 32:160]
        bd = cb[:, 160:288]
        zs = cb[:, 288:416]
        egrp = cb[0:GROUPS, 416:544]        # E[g, 32g'+k] = (g == g')
        iota = cb[:, 544:545]               # partition index % 32

        yt8 = singles.tile([GROUPS, MCOL], i8)
        nc.scalar.dma_start(out=yt8, in_=ALLd.ap()[0:32, C_YT:C_YT + YW])
        yt = singles.tile([GROUPS, MCOL], bf16)
        for j in range(4):
            ysl = slice(j * (MCOL // 4), (j + 1) * (MCOL // 4))
            nc.scalar.activation(yt[:, ysl], yt8[:, ysl], AF.Copy)
        # one-hot per timestep: block j holds onehot(y at t=j-1); block 0 = 0
        ohbuf = singles.tile([128, (L + 1) * GB], bf16)
        nc.vector.memset(ohbuf[:, 0:GB], 0.0)

        expsc = singles.tile([128, MCOL], bf16)

        nshift = singles.tile([128, 1], f32)
        nc.vector.memset(nshift, -SHIFT)
        ones = singles.tile([128, 1], f32)
        nc.vector.memset(ones, 1.0)
        goldacc = singles.tile([128, NT], f32)
        logacc = singles.tile([128, 16], f32)
        nc.vector.memset(logacc, 0.0)
        combo = singles.tile([128, 2], f32)

        with tc.tile_pool(name="x8", bufs=3) as x8p, \
             tc.tile_pool(name="xt", bufs=3) as xtp, \
             tc.tile_pool(name="scp", bufs=2, space="PSUM") as scp, \
             tc.tile_pool(name="yrp", bufs=2, space="PSUM") as yrp, \
             tc.tile_pool(name="mp", bufs=2) as mp, \
             tc.tile_pool(name="up", bufs=2, space="PSUM") as up, \
             tc.tile_pool(name="vp", bufs=2) as vp, \
             tc.tile_pool(name="rzp", bufs=2) as rzp, \
             tc.tile_pool(name="lnp", bufs=2) as lnp, \
             tc.tile_pool(name="pp", bufs=2) as pp:

            p_prev = None
            nidx = 0

            def recursion_step(t):
                nonlocal p_prev, nidx
                ctx2 = tc.high_priority()
                ctx2.__enter__()
                u = up.tile([128, 2 * GB], f32, tag="u")
                nc.tensor.matmul(u[:, 0:GB], lhsT=bd, rhs=p_prev,
                                 start=True, stop=True)
                i = t // 2
                e_sl = expsc[:, i * TILE_COLS + (t % 2) * GB:
                             i * TILE_COLS + (t % 2) * GB + GB]
                if t % 3 == 0:
                    v = vp.tile([128, GB], bf16)
                    nc.vector.tensor_mul(v, u[:, 0:GB], e_sl)
                    z = u[:, GB:2 * GB]
                    nc.tensor.matmul(z, lhsT=zs, rhs=v, start=True, stop=True,
                                     skip_group_check=True)
                    rz = rzp.tile([128, GB], f32)
                    nc.vector.reciprocal(rz, z)
                    lnscr = lnp.tile([128, GB], bf16)
                    nc.scalar.activation(lnscr, z, AF.Ln,
                                         accum_out=logacc[:, nidx:nidx + 1])
                    nidx += 1
                    pn = pp.tile([128, GB], bf16)
                    nc.vector.tensor_mul(pn, v, rz)
                else:
                    pn = pp.tile([128, GB], bf16)
                    nc.vector.tensor_mul(pn, u[:, 0:GB], e_sl)
                ctx2.__exit__(None, None, None)
                p_prev = pn

            for i in range(NT):
                cs = slice(i * TILE_COLS, (i + 1) * TILE_COLS)

                x8i = x8p.tile([128, GROUPS * TILE_COLS], i8)
                nc.sync.dma_start(
                    out=x8i,
                    in_=ALLd.ap()[:, i * GROUPS * TILE_COLS:
                                  (i + 1) * GROUPS * TILE_COLS])
                xti = xtp.tile([128, GROUPS * TILE_COLS], bf16)
                nc.scalar.activation(xti, x8i, AF.Copy, scale=float(XSCALE))

                # one-hot masks: write blocks for t=2i, 2i+1; the prev-label
                # mask is the same buffer shifted one timestep back
                yrep = yrp.tile([128, TILE_COLS], f32, tag="yr")
                nc.tensor.matmul(yrep, lhsT=egrp, rhs=yt[:, cs],
                                 start=True, stop=True)
                nc.vector.tensor_tensor(
                    ohbuf[:, (2 * i + 1) * GB:(2 * i + 3) * GB], yrep,
                    iota.to_broadcast([128, TILE_COLS]), ALU.is_equal)
                oh_t = ohbuf[:, (2 * i + 1) * GB:(2 * i + 3) * GB]
                ohp_t = ohbuf[:, (2 * i) * GB:(2 * i + 2) * GB]

                sc = scp.tile([128, TILE_COLS], f32)
                for g in range(GROUPS):
                    nc.tensor.matmul(
                        sc[32 * g:32 * g + 32, :],
                        lhsT=wblk,
                        rhs=xti[:, g * TILE_COLS:(g + 1) * TILE_COLS],
                        start=True, stop=True,
                        tile_position=(0, 32 * g),
                    )
                nc.scalar.activation(expsc[:, cs], sc, AF.Exp,
                                     bias=nshift[:, 0:1])

                # accumulate T[y_prev, k] into the score psum (after exp read)
                nc.tensor.matmul(sc, lhsT=tb, rhs=ohp_t,
                                 start=False, stop=True,
                                 skip_group_check=True)

                m1 = mp.tile([128, TILE_COLS], f32)
                nc.vector.tensor_tensor(m1, sc, oh_t, ALU.mult)
                nc.vector.tensor_reduce(goldacc[:, i:i + 1], m1,
                                        axis=X_AX, op=ALU.add)

                # recursion steps enabled by this tile
                if i == 0:
                    p_prev = expsc[:, 0:GB]
                    recursion_step(1)
                else:
                    recursion_step(2 * i)
                    recursion_step(2 * i + 1)

            # final partition-function sum (t = L-1 state)
            zf = up.tile([128, 2 * GB], f32, tag="u")
            nc.tensor.matmul(zf[:, 0:GB], lhsT=zs, rhs=p_prev,
                             start=True, stop=True)
            lnscr = lnp.tile([128, GB], bf16)
            nc.scalar.activation(lnscr, zf[:, 0:GB], AF.Ln,
                                 accum_out=logacc[:, nidx:nidx + 1])
            nidx += 1
            assert nidx == 11

            nc.vector.tensor_reduce(combo[:, 0:1], logacc, axis=X_AX,
                                    op=ALU.add)
            nc.vector.tensor_reduce(combo[:, 1:2], goldacc, axis=X_AX,
                                    op=ALU.add)
            res_ps = up.tile([128, 2 * GB], f32, tag="u")
            nc.tensor.matmul(res_ps[0:2, 0:1], lhsT=combo, rhs=ones,
                             start=True, stop=True)
            outsb = singles.tile([2, 1], f32)
            nc.vector.tensor_copy(out=outsb, in_=res_ps[0:2, 0:1])
            nc.sync.dma_start(out=OUTd.ap(), in_=outsb)

    nc.compile()
    return nc


def _get_program():
    if "nc" not in _cache:
        _cache["nc"] = _build_program()
    return _cache["nc"]


def _pack_labels(lab):
    """[4, 8192] int8 label rows -> [32, 1024] packed (1 KB DMA lines) so a
    DMA from rows 0-31 of the label region into [4, 8192] reproduces them."""
    flat = lab.reshape(-1)                       # j = p*8192 + c
    return flat.reshape(32, YW)                  # row p' holds j = p'*1024 ...


def _make_consts(W, T, bd_scale):
    import ml_dtypes
    bf = ml_dtypes.bfloat16
    cb = np.zeros((128, CBW), dtype=np.float64)
    cb[:, :K] = W.astype(bf).astype(np.float64)
    expTs = np.exp(T.astype(np.float64)) / bd_scale
    Tb = T.astype(bf).astype(np.float64)
    for g in range(GROUPS):
        r = slice(32 * g, 32 * g + K)
        cb[r, 32 + 32 * g:32 + 32 * g + K] = Tb       # tb (blockdiag T)
        cb[r, 160 + 32 * g:160 + 32 * g + K] = expTs  # bd (blockdiag expT/s)
        cb[g, 416 + 32 * g:416 + 32 * g + 32] = 1     # egrp (group select)
    for r in range(128):
        g = r // 32
        if r % 32 < K:
            cb[r, 288 + 32 * g:288 + 32 * g + 32] = 1  # zs (group-sum)
        cb[r, 544] = r % 32                            # iota (label index)
    # two-plane int8 decomposition: cb ~= A + B/128
    A = np.clip(np.round(cb), -127, 127)
    Bp = np.clip(np.round((cb - A) * 128.0), -127, 127)
    return A.astype(np.int8), Bp.astype(np.int8)


def _make_in_maps(X, y, W, T):
    W = np.asarray(W)
    T = np.asarray(T)
    bd_scale = max(1.0, float(np.exp(T.astype(np.float64)).max()) / 120.0)
    _cache["bd_scale"] = bd_scale
    cba, cbb = _make_consts(W, T, bd_scale)

    X = np.asarray(X, dtype=np.float32)
    y = np.asarray(y)
    in_maps = []
    for cidx in range(N_CORES):
        Xc = X[cidx * BC:(cidx + 1) * BC]               # [1024, 32, 128]
        Xg = Xc.reshape(GROUPS, GB, L, F)               # [g, b, t, f]
        # X cols = (tile, group, t_local, b): i*2048 + g*512 + tl*256 + b
        XT = (Xg.transpose(3, 2, 0, 1)                  # [f, t, g, b]
                .reshape(F, NT, 2, GROUPS, GB)          # [f, i, tl, g, b]
                .transpose(0, 1, 3, 2, 4)               # [f, i, g, tl, b]
                .reshape(F, XCOL))
        Xq = np.clip(np.round(XT / XSCALE), -127, 127).astype(np.int8)

        Yc = y[cidx * BC:(cidx + 1) * BC].astype(np.int64)  # [1024, 32]
        Yg = Yc.reshape(GROUPS, GB, L)                  # [g, b, t]

        # label rows: [g, (tile, t_local, b)] = [4, i*512 + tl*256 + b]
        def lrows(lbl):
            return (lbl.transpose(0, 2, 1)              # [g, t, b]
                       .reshape(GROUPS, MCOL).astype(np.int8))

        allt = np.zeros((128, WT), dtype=np.int8)
        allt[:, :XCOL] = Xq
        allt[0:32, C_YT:C_YT + YW] = _pack_labels(lrows(Yg))
        allt[:, C_CBA:C_CBA + CBW] = cba
        allt[:, C_CBB:C_CBB + CBW] = cbb
        in_maps.append({"ALL": allt})
    return in_maps


def _combine(results):
    bd_scale = _cache.get("bd_scale", 1.0)
    lncorr = BC * (L - 1) * np.log(bd_scale)
    total = 0.0
    for r in results:
        o = np.asarray(r["out"], dtype=np.float64).reshape(-1)
        sumlog = o[0] / 32.0
        gold = o[1]
        total += gold - (sumlog + lncorr) - BC * L * SHIFT
    return np.float32(total / B)


def kernel(X, y, W, T):
    from concourse.bass_utils import run_bass_kernel_spmd
    nc = _get_program()
    in_maps = _make_in_maps(np.asarray(X), np.asarray(y),
                            np.asarray(W), np.asarray(T))
    res = run_bass_kernel_spmd(nc, in_maps, list(range(N_CORES)))
    return _combine(res.results)


# revision 16
# speedup vs baseline: 1.3670x; 1.1085x over previous
"""CRF loss kernel for Trainium2 (8 NeuronCores, data-parallel over batch).

Reference computation (see problem):
    score = einsum('blf,fk->blk', X, W);  forward/backward CRF messages over L;
    loss = mean_b(emit + trans - logZ).

The per-exec harness cost is dominated by per-tensor staging overhead, so ALL
inputs ship as ONE int8 tensor per core (~4.2 MiB):
  cols [0, 32768)      X as fp8-e3m4 BIT PATTERNS (int8 dtype keeps the fast
                       staging path), X^T in [F=128, (tile, group, t, b)] layout
  cols [32768, 33792)  YT labels y[b,t] packed [4,8192]->[32,1024] (1KB lines)
  cols [33792, 34340)  A-plane of bf16 consts CB (round(c))
  cols [34340, 34888)  B-plane (round((c-A)*128)); device cb = A + B/128
CB = [W pad | blockdiag(T) | blockdiag(expT/s) | group-sum | group-select E |
label iota]; s rescales expT into int8-decomp range and is corrected on host.

Device algorithm (per core, batch shard of 1024):
  - decode consts + label rows; X tiles are DMA'd as raw bytes and the PE
    reads them as fp8 directly via AP bitcast (no decode pass).
  - 16 pipelined tiles of 512 columns (2 timesteps x 256 batch, 4 batch
    groups packed on partitions at offsets 0/32/64/96):
      score psum = W^T @ XT tile (4 matmuls, tile_position packing)
      expsc = exp(score - SHIFT) via ACT (fused PSUM->SBUF), bf16
      masks on device: yrep = E^T @ YT row (partition-broadcast), then
      oh = is_equal(yrep, iota) on DVE into a per-timestep buffer whose
      one-block-shifted slice doubles as the prev-label mask
      tcol = blockdiag(T)^T @ ohp accumulated INTO the score psum after the
      exp read; gold tile sum = reduce((score + tcol) .* oh)
  - CRF forward recursion in probability domain, interleaved with the tile
    loop: p_t = (BD^T @ p_{t-1}) * expsc_t, renormalized every 3 steps by
    group-sum Z, accumulating log Z via ACT Ln accum_out.
  - out[2,1]: [32*sum_b sum log Z, emit+trans total]
Host: loss = sum_cores(gold - sumlog/32 - BC*L*SHIFT - BC*(L-1)*ln s) / B.
"""

import numpy as np

B, L, F, K = 8192, 32, 128, 26
N_CORES = 8
BC = B // N_CORES            # batch per core
GROUPS = 4                   # batch groups packed on partition blocks
GB = BC // GROUPS            # 256 batch columns per group
NT = L // 2                  # 16 tiles, 2 timesteps each
TILE_COLS = 2 * GB           # 512 columns per tile
SHIFT = 26.0

XCOL = NT * TILE_COLS * GROUPS          # 32768 X-code columns
MCOL = NT * TILE_COLS                   # 8192 mask/expsc columns
YW = 1024                              # label region cols (rows 0-31 used)
CBW = 548                               # const columns
C_YT = XCOL
C_CBA = C_YT + YW
C_CBB = C_CBA + CBW
WT = C_CBB + CBW                        # 34120 total columns

_cache = {}


def _build_program():
    import concourse.bass as bass  # noqa: F401
    import concourse.bacc as bacc
    import concourse.tile as tile
    from concourse import mybir
    from contextlib import ExitStack

    f32 = mybir.dt.float32
    bf16 = mybir.dt.bfloat16
    i8 = mybir.dt.int8
    fp8 = mybir.dt.float8e3
    AF = mybir.ActivationFunctionType
    ALU = mybir.AluOpType
    X_AX = mybir.AxisListType.X

    nc = bacc.Bacc("TRN2", target_bir_lowering=False)

    ALLd = nc.dram_tensor("ALL", [128, WT], i8, kind="ExternalInput")
    OUTd = nc.dram_tensor("out", [2, 1], f32, kind="ExternalOutput")

    with tile.TileContext(nc) as tc, ExitStack() as ctx:
        singles = ctx.enter_context(tc.tile_pool(name="singles", bufs=1))

        # ---- const + label decode ----
        cba8 = singles.tile([128, CBW], i8)
        nc.sync.dma_start(out=cba8, in_=ALLd.ap()[:, C_CBA:C_CBB])
        cbb8 = singles.tile([128, CBW], i8)
        nc.sync.dma_start(out=cbb8, in_=ALLd.ap()[:, C_CBB:C_CBB + CBW])
        cbtmp = singles.tile([128, CBW], bf16)
        nc.vector.tensor_scalar(cbtmp, cbb8, scalar1=1.0 / 128.0, scalar2=None,
                                op0=ALU.mult)
        cba = singles.tile([128, CBW], bf16)
        nc.vector.tensor_copy(out=cba, in_=cba8)
        cb = singles.tile([128, CBW], bf16)
        nc.vector.tensor_tensor(cb, cba, cbtmp, ALU.add)

        wblk = cb[:, 0:32]
        tb = cb[:, 32:160]
        bd = cb[:, 160:288]
        zs = cb[:, 288:416]
        egrp = cb[0:GROUPS, 416:544]        # E[g, 32g'+k] = (g == g')
        iota = cb[:, 544:545]               # partition index % 32

        yt8 = singles.tile([GROUPS, MCOL], i8)
        nc.scalar.dma_start(out=yt8, in_=ALLd.ap()[0:32, C_YT:C_YT + YW])
        yt = singles.tile([GROUPS, MCOL], bf16)
        for j in range(4):
            ysl = slice(j * (MCOL // 4), (j + 1) * (MCOL // 4))
            nc.scalar.activation(yt[:, ysl], yt8[:, ysl], AF.Copy)
        # one-hot per timestep: block j holds onehot(y at t=j-1); block 0 = 0
        ohbuf = singles.tile([128, (L + 1) * GB], bf16)
        nc.vector.memset(ohbuf[:, 0:GB], 0.0)

        expsc = singles.tile([128, MCOL], bf16)

        nshift = singles.tile([128, 1], f32)
        nc.vector.memset(nshift, -SHIFT)
        ones = singles.tile([128, 1], f32)
        nc.vector.memset(ones, 1.0)
        goldacc = singles.tile([128, NT], f32)
        nc.vector.memset(goldacc, 0.0)
        logacc = singles.tile([128, 16], f32)
        nc.vector.memset(logacc, 0.0)
        combo = singles.tile([128, 2], f32)

        with tc.tile_pool(name="x8", bufs=3) as x8p, \
             tc.tile_pool(name="scp", bufs=2, space="PSUM") as scp, \
             tc.tile_pool(name="yrp", bufs=2, space="PSUM") as yrp, \
             tc.tile_pool(name="mp", bufs=2) as mp, \
             tc.tile_pool(name="up", bufs=2, space="PSUM") as up, \
             tc.tile_pool(name="vp", bufs=2) as vp, \
             tc.tile_pool(name="rzp", bufs=2) as rzp, \
             tc.tile_pool(name="lnp", bufs=2) as lnp, \
             tc.tile_pool(name="pp", bufs=2) as pp:

            p_prev = None
            nidx = 0

            def recursion_step(t):
                nonlocal p_prev, nidx
                ctx2 = tc.high_priority()
                ctx2.__enter__()
                u = up.tile([128, 2 * GB], f32, tag="u")
                nc.tensor.matmul(u[:, 0:GB], lhsT=bd, rhs=p_prev,
                                 start=True, stop=True)
                i = t // 2
                e_sl = expsc[:, i * TILE_COLS + (t % 2) * GB:
                             i * TILE_COLS + (t % 2) * GB + GB]
                if t % 3 == 0:
                    v = vp.tile([128, GB], bf16)
                    nc.vector.tensor_mul(v, u[:, 0:GB], e_sl)
                    z = u[:, GB:2 * GB]
                    nc.tensor.matmul(z, lhsT=zs, rhs=v, start=True, stop=True,
                                     skip_group_check=True)
                    rz = rzp.tile([128, GB], f32)
                    nc.vector.reciprocal(rz, z)
                    lnscr = lnp.tile([128, GB], bf16)
                    nc.scalar.activation(lnscr, z, AF.Ln,
                                         accum_out=logacc[:, nidx:nidx + 1])
                    nidx += 1
                    pn = pp.tile([128, GB], bf16)
                    nc.vector.tensor_mul(pn, v, rz)
                else:
                    pn = pp.tile([128, GB], bf16)
                    nc.vector.tensor_mul(pn, u[:, 0:GB], e_sl)
                ctx2.__exit__(None, None, None)
                p_prev = pn

            for i in range(NT):
                cs = slice(i * TILE_COLS, (i + 1) * TILE_COLS)

                x8i = x8p.tile([128, GROUPS * TILE_COLS], i8)
                nc.sync.dma_start(
                    out=x8i,
                    in_=ALLd.ap()[:, i * GROUPS * TILE_COLS:
                                  (i + 1) * GROUPS * TILE_COLS])
                xti = x8i.bitcast(fp8)

                # one-hot masks: write blocks for t=2i, 2i+1; the prev-label
                # mask is the same buffer shifted one timestep back
                yrep = yrp.tile([128, TILE_COLS], f32, tag="yr")
                nc.tensor.matmul(yrep, lhsT=egrp, rhs=yt[:, cs],
                                 start=True, stop=True)
                nc.vector.tensor_tensor(
                    ohbuf[:, (2 * i + 1) * GB:(2 * i + 3) * GB], yrep,
                    iota.to_broadcast([128, TILE_COLS]), ALU.is_equal)
                oh_t = ohbuf[:, (2 * i + 1) * GB:(2 * i + 3) * GB]
                ohp_t = ohbuf[:, (2 * i) * GB:(2 * i + 2) * GB]

                sc = scp.tile([128, TILE_COLS], f32)
                for g in range(GROUPS):
                    nc.tensor.matmul(
                        sc[32 * g:32 * g + 32, :],
                        lhsT=wblk,
                        rhs=xti[:, g * TILE_COLS:(g + 1) * TILE_COLS],
                        start=True, stop=True,
                        tile_position=(0, 32 * g),
                    )
                nc.scalar.activation(expsc[:, cs], sc, AF.Exp,
                                     bias=nshift[:, 0:1])

                # accumulate T[y_prev, k] into the score psum (after exp read)
                nc.tensor.matmul(sc, lhsT=tb, rhs=ohp_t,
                                 start=False, stop=True,
                                 skip_group_check=True)

                m1 = mp.tile([128, TILE_COLS], f32)
                nc.vector.tensor_tensor(m1, sc, oh_t, ALU.mult)
                mscr = mp.tile([128, TILE_COLS], f32, tag="mscr")
                nc.scalar.activation(mscr, m1, AF.Copy,
                                     accum_out=goldacc[:, i:i + 1])

                # recursion steps enabled by this tile
                if i == 0:
                    p_prev = expsc[:, 0:GB]
                    recursion_step(1)
                else:
                    recursion_step(2 * i)
                    recursion_step(2 * i + 1)

            # final partition-function sum (t = L-1 state)
            zf = up.tile([128, 2 * GB], f32, tag="u")
            nc.tensor.matmul(zf[:, 0:GB], lhsT=zs, rhs=p_prev,
                             start=True, stop=True)
            lnscr = lnp.tile([128, GB], bf16)
            nc.scalar.activation(lnscr, zf[:, 0:GB], AF.Ln,
                                 accum_out=logacc[:, nidx:nidx + 1])
            nidx += 1
            assert nidx == 11

            nc.vector.tensor_reduce(combo[:, 0:1], logacc, axis=X_AX,
                                    op=ALU.add)
            nc.vector.tensor_reduce(combo[:, 1:2], goldacc, axis=X_AX,
                                    op=ALU.add)
            res_ps = up.tile([128, 2 * GB], f32, tag="u")
            nc.tensor.matmul(res_ps[0:2, 0:1], lhsT=combo, rhs=ones,
                             start=True, stop=True)
            outsb = singles.tile([2, 1], f32)
            nc.vector.tensor_copy(out=outsb, in_=res_ps[0:2, 0:1])
            nc.sync.dma_start(out=OUTd.ap(), in_=outsb)

    nc.compile()
    return nc


def _get_program():
    if "nc" not in _cache:
        _cache["nc"] = _build_program()
    return _cache["nc"]


def _pack_labels(lab):
    """[4, 8192] int8 label rows -> [32, 1024] packed (1 KB DMA lines) so a
    DMA from rows 0-31 of the label region into [4, 8192] reproduces them."""
    flat = lab.reshape(-1)                       # j = p*8192 + c
    return flat.reshape(32, YW)                  # row p' holds j = p'*1024 ...


def _make_consts(W, T, bd_scale):
    import ml_dtypes
    bf = ml_dtypes.bfloat16
    cb = np.zeros((128, CBW), dtype=np.float64)
    cb[:, :K] = W.astype(bf).astype(np.float64)
    expTs = np.exp(T.astype(np.float64)) / bd_scale
    Tb = T.astype(bf).astype(np.float64)
    for g in range(GROUPS):
        r = slice(32 * g, 32 * g + K)
        cb[r, 32 + 32 * g:32 + 32 * g + K] = Tb       # tb (blockdiag T)
        cb[r, 160 + 32 * g:160 + 32 * g + K] = expTs  # bd (blockdiag expT/s)
        cb[g, 416 + 32 * g:416 + 32 * g + 32] = 1     # egrp (group select)
    for r in range(128):
        g = r // 32
        if r % 32 < K:
            cb[r, 288 + 32 * g:288 + 32 * g + 32] = 1  # zs (group-sum)
        cb[r, 544] = r % 32                            # iota (label index)
    # two-plane int8 decomposition: cb ~= A + B/128
    A = np.clip(np.round(cb), -127, 127)
    Bp = np.clip(np.round((cb - A) * 128.0), -127, 127)
    return A.astype(np.int8), Bp.astype(np.int8)


def _make_in_maps(X, y, W, T):
    import ml_dtypes
    W = np.asarray(W)
    T = np.asarray(T)
    bd_scale = max(1.0, float(np.exp(T.astype(np.float64)).max()) / 120.0)
    _cache["bd_scale"] = bd_scale
    cba, cbb = _make_consts(W, T, bd_scale)

    X = np.asarray(X, dtype=np.float32)
    y = np.asarray(y)
    in_maps = []
    for cidx in range(N_CORES):
        Xc = X[cidx * BC:(cidx + 1) * BC]               # [1024, 32, 128]
        Xg = Xc.reshape(GROUPS, GB, L, F)               # [g, b, t, f]
        # X cols = (tile, group, t_local, b): i*2048 + g*512 + tl*256 + b
        XT = (Xg.transpose(3, 2, 0, 1)                  # [f, t, g, b]
                .reshape(F, NT, 2, GROUPS, GB)          # [f, i, tl, g, b]
                .transpose(0, 1, 3, 2, 4)               # [f, i, g, tl, b]
                .reshape(F, XCOL))
        Xq = np.ascontiguousarray(XT).astype(ml_dtypes.float8_e3m4).view(np.int8)

        Yc = y[cidx * BC:(cidx + 1) * BC].astype(np.int64)  # [1024, 32]
        Yg = Yc.reshape(GROUPS, GB, L)                  # [g, b, t]

        # label rows: [g, (tile, t_local, b)] = [4, i*512 + tl*256 + b]
        def lrows(lbl):
            return (lbl.transpose(0, 2, 1)              # [g, t, b]
                       .reshape(GROUPS, MCOL).astype(np.int8))

        allt = np.zeros((128, WT), dtype=np.int8)
        allt[:, :XCOL] = Xq
        allt[0:32, C_YT:C_YT + YW] = _pack_labels(lrows(Yg))
        allt[:, C_CBA:C_CBA + CBW] = cba
        allt[:, C_CBB:C_CBB + CBW] = cbb
        in_maps.append({"ALL": allt})
    return in_maps


def _combine(results):
    bd_scale = _cache.get("bd_scale", 1.0)
    lncorr = BC * (L - 1) * np.log(bd_scale)
    total = 0.0
    for r in results:
        o = np.asarray(r["out"], dtype=np.float64).reshape(-1)
        sumlog = o[0] / 32.0
        gold = o[1]
        total += gold - (sumlog + lncorr) - BC * L * SHIFT
    return np.float32(total / B)


def kernel(X, y, W, T):
    from concourse.bass_utils import run_bass_kernel_spmd
    nc = _get_program()
    in_maps = _make_in_maps(np.asarray(X), np.asarray(y),
                            np.asarray(W), np.asarray(T))
    res = run_bass_kernel_spmd(nc, in_maps, list(range(N_CORES)))
    return _combine(res.results)


# revision 18
# speedup vs baseline: 1.4215x; 1.0399x over previous
"""CRF loss kernel for Trainium2 (8 NeuronCores, data-parallel over batch).

Reference computation (see problem):
    score = einsum('blf,fk->blk', X, W);  forward/backward CRF messages over L;
    loss = mean_b(emit + trans - logZ).

The per-exec harness cost is dominated by per-tensor staging overhead, so ALL
inputs ship as ONE int8 tensor per core (~4.2 MiB):
  cols [0, 32768)      X as fp8-e3m4 BIT PATTERNS (int8 dtype keeps the fast
                       staging path), X^T in [F=128, (tile, group, t, b)] layout
  cols [32768, 33792)  YT labels y[b,t] packed [4,8192]->[32,1024] (1KB lines)
  cols [33792, 34340)  A-plane of bf16 consts CB (round(c))
  cols [34340, 34888)  B-plane (round((c-A)*128)); device cb = A + B/128
CB = [W pad | blockdiag(T) | blockdiag(expT/s) | group-sum | group-select E |
label iota]; s rescales expT into int8-decomp range and is corrected on host.

Device algorithm (per core, batch shard of 1024):
  - decode consts + label rows; X tiles are DMA'd as raw bytes and the PE
    reads them as fp8 directly via AP bitcast (no decode pass).
  - 16 pipelined tiles of 512 columns (2 timesteps x 256 batch, 4 batch
    groups packed on partitions at offsets 0/32/64/96):
      score psum = W^T @ XT tile (4 matmuls, tile_position packing)
      expsc = exp(score - SHIFT) via ACT (fused PSUM->SBUF), bf16
      masks on device: yrep = E^T @ YT row (partition-broadcast), then
      oh = is_equal(yrep, iota) on DVE into a per-timestep buffer whose
      one-block-shifted slice doubles as the prev-label mask
      tcol = blockdiag(T)^T @ ohp accumulated INTO the score psum after the
      exp read; gold tile sum = reduce((score + tcol) .* oh)
  - CRF forward recursion in probability domain, interleaved with the tile
    loop: p_t = (BD^T @ p_{t-1}) * expsc_t, renormalized every 3 steps by
    group-sum Z, accumulating log Z via ACT Ln accum_out.
  - out[2,1]: [32*sum_b sum log Z, emit+trans total]
Host: loss = sum_cores(gold - sumlog/32 - BC*L*SHIFT - BC*(L-1)*ln s) / B.
"""

import numpy as np

B, L, F, K = 8192, 32, 128, 26
N_CORES = 8
BC = B // N_CORES            # batch per core
GROUPS = 4                   # batch groups packed on partition blocks
GB = BC // GROUPS            # 256 batch columns per group
NT = L // 2                  # 16 tiles, 2 timesteps each
TILE_COLS = 2 * GB           # 512 columns per tile
SHIFT = 26.0

XCOL = NT * TILE_COLS * GROUPS          # 32768 X-code columns
MCOL = NT * TILE_COLS                   # 8192 mask/expsc columns
YW = 1024                              # label region cols (rows 0-31 used)
CBW = 548                               # const columns
C_YT = XCOL
C_CBA = C_YT + YW
C_CBB = C_CBA + CBW
WT = C_CBB + CBW                        # 34120 total columns

_cache = {}


def _build_program():
    import concourse.bass as bass  # noqa: F401
    import concourse.bacc as bacc
    import concourse.tile as tile
    from concourse import mybir
    from contextlib import ExitStack

    f32 = mybir.dt.float32
    bf16 = mybir.dt.bfloat16
    i8 = mybir.dt.int8
    fp8 = mybir.dt.float8e3
    AF = mybir.ActivationFunctionType
    ALU = mybir.AluOpType
    X_AX = mybir.AxisListType.X

    nc = bacc.Bacc("TRN2", target_bir_lowering=False)

    ALLd = nc.dram_tensor("ALL", [128, WT], i8, kind="ExternalInput")
    OUTd = nc.dram_tensor("out", [2, 1], f32, kind="ExternalOutput")

    with tile.TileContext(nc) as tc, ExitStack() as ctx:
        singles = ctx.enter_context(tc.tile_pool(name="singles", bufs=1))

        # ---- const + label decode ----
        cba8 = singles.tile([128, CBW], i8)
        nc.sync.dma_start(out=cba8, in_=ALLd.ap()[:, C_CBA:C_CBB])
        cbb8 = singles.tile([128, CBW], i8)
        nc.sync.dma_start(out=cbb8, in_=ALLd.ap()[:, C_CBB:C_CBB + CBW])
        cbtmp = singles.tile([128, CBW], bf16)
        nc.vector.tensor_scalar(cbtmp, cbb8, scalar1=1.0 / 128.0, scalar2=None,
                                op0=ALU.mult)
        cba = singles.tile([128, CBW], bf16)
        nc.vector.tensor_copy(out=cba, in_=cba8)
        cb = singles.tile([128, CBW], bf16)
        nc.vector.tensor_tensor(cb, cba, cbtmp, ALU.add)

        wblk = cb[:, 0:32]
        tb = cb[:, 32:160]
        bd = cb[:, 160:288]
        zs = cb[:, 288:416]
        egrp = cb[0:GROUPS, 416:544]        # E[g, 32g'+k] = (g == g')
        iota = cb[:, 544:545]               # partition index % 32

        yt8 = singles.tile([GROUPS, MCOL], i8)
        nc.scalar.dma_start(out=yt8, in_=ALLd.ap()[0:32, C_YT:C_YT + YW])
        yt = singles.tile([GROUPS, MCOL], bf16)
        for j in range(4):
            ysl = slice(j * (MCOL // 4), (j + 1) * (MCOL // 4))
            nc.scalar.activation(yt[:, ysl], yt8[:, ysl], AF.Copy)
        # one-hot per timestep: block j holds onehot(y at t=j-1); block 0 = 0
        ohbuf = singles.tile([128, (L + 1) * GB], bf16)
        nc.vector.memset(ohbuf[:, 0:GB], 0.0)

        expsc = singles.tile([128, MCOL], bf16)

        nshift = singles.tile([128, 1], f32)
        nc.vector.memset(nshift, -SHIFT)
        ones = singles.tile([128, 1], f32)
        nc.vector.memset(ones, 1.0)
        goldacc = singles.tile([128, NT], f32)
        nc.vector.memset(goldacc, 0.0)
        logacc = singles.tile([128, 16], f32)
        nc.vector.memset(logacc, 0.0)
        combo = singles.tile([128, 2], f32)

        with tc.tile_pool(name="x8", bufs=3) as x8p, \
             tc.tile_pool(name="scp", bufs=2, space="PSUM") as scp, \
             tc.tile_pool(name="yrp", bufs=2, space="PSUM") as yrp, \
             tc.tile_pool(name="mp", bufs=2) as mp, \
             tc.tile_pool(name="up", bufs=2, space="PSUM") as up, \
             tc.tile_pool(name="vp", bufs=2) as vp, \
             tc.tile_pool(name="rzp", bufs=2) as rzp, \
             tc.tile_pool(name="lnp", bufs=2) as lnp, \
             tc.tile_pool(name="pp", bufs=2) as pp:

            p_prev = None
            nidx = 0

            def recursion_step(t):
                nonlocal p_prev, nidx
                ctx2 = tc.high_priority()
                ctx2.__enter__()
                u = up.tile([128, 2 * GB], f32, tag="u")
                nc.tensor.matmul(u[:, 0:GB], lhsT=bd, rhs=p_prev,
                                 start=True, stop=True)
                i = t // 2
                e_sl = expsc[:, i * TILE_COLS + (t % 2) * GB:
                             i * TILE_COLS + (t % 2) * GB + GB]
                if t % 3 == 0:
                    v = vp.tile([128, GB], bf16)
                    nc.vector.tensor_mul(v, u[:, 0:GB], e_sl)
                    z = u[:, GB:2 * GB]
                    nc.tensor.matmul(z, lhsT=zs, rhs=v, start=True, stop=True,
                                     skip_group_check=True)
                    rz = rzp.tile([128, GB], f32)
                    nc.vector.reciprocal(rz, z)
                    lnscr = lnp.tile([128, GB], bf16)
                    nc.scalar.activation(lnscr, z, AF.Ln,
                                         accum_out=logacc[:, nidx:nidx + 1])
                    nidx += 1
                    pn = pp.tile([128, GB], bf16)
                    nc.vector.tensor_mul(pn, v, rz)
                else:
                    pn = pp.tile([128, GB], bf16)
                    nc.vector.tensor_mul(pn, u[:, 0:GB], e_sl)
                ctx2.__exit__(None, None, None)
                p_prev = pn

            for i in range(NT):
                cs = slice(i * TILE_COLS, (i + 1) * TILE_COLS)

                x8i = x8p.tile([128, GROUPS * TILE_COLS], i8)
                nc.sync.dma_start(
                    out=x8i,
                    in_=ALLd.ap()[:, i * GROUPS * TILE_COLS:
                                  (i + 1) * GROUPS * TILE_COLS])
                xti = x8i.bitcast(fp8)

                # one-hot masks: write blocks for t=2i, 2i+1; the prev-label
                # mask is the same buffer shifted one timestep back
                yrep = yrp.tile([128, TILE_COLS], f32, tag="yr")
                nc.tensor.matmul(yrep, lhsT=egrp, rhs=yt[:, cs],
                                 start=True, stop=True)
                nc.vector.tensor_tensor(
                    ohbuf[:, (2 * i + 1) * GB:(2 * i + 3) * GB], yrep,
                    iota.to_broadcast([128, TILE_COLS]), ALU.is_equal)
                oh_t = ohbuf[:, (2 * i + 1) * GB:(2 * i + 3) * GB]
                ohp_t = ohbuf[:, (2 * i) * GB:(2 * i + 2) * GB]

                sc = scp.tile([128, TILE_COLS], f32)
                for g in range(GROUPS):
                    nc.tensor.matmul(
                        sc[32 * g:32 * g + 32, :],
                        lhsT=wblk,
                        rhs=xti[:, g * TILE_COLS:(g + 1) * TILE_COLS],
                        start=True, stop=True,
                        tile_position=(0, 32 * g),
                    )
                nc.scalar.activation(expsc[:, cs], sc, AF.Exp,
                                     bias=nshift[:, 0:1])

                # accumulate T[y_prev, k] into the score psum (after exp read)
                nc.tensor.matmul(sc, lhsT=tb, rhs=ohp_t,
                                 start=False, stop=True,
                                 skip_group_check=True)

                m1 = mp.tile([128, TILE_COLS], f32)
                nc.vector.tensor_tensor(m1, sc, oh_t, ALU.mult)
                mscr = mp.tile([128, TILE_COLS], f32, tag="mscr")
                nc.scalar.activation(mscr, m1, AF.Copy,
                                     accum_out=goldacc[:, i:i + 1])

                # recursion steps enabled by this tile
                if i == 0:
                    p_prev = expsc[:, 0:GB]
                    recursion_step(1)
                else:
                    recursion_step(2 * i)
                    recursion_step(2 * i + 1)

            # final partition-function sum (t = L-1 state)
            zf = up.tile([128, 2 * GB], f32, tag="u")
            nc.tensor.matmul(zf[:, 0:GB], lhsT=zs, rhs=p_prev,
                             start=True, stop=True)
            lnscr = lnp.tile([128, GB], bf16)
            nc.scalar.activation(lnscr, zf[:, 0:GB], AF.Ln,
                                 accum_out=logacc[:, nidx:nidx + 1])
            nidx += 1
            assert nidx == 11

            nc.vector.tensor_reduce(combo[:, 0:1], logacc, axis=X_AX,
                                    op=ALU.add)
            nc.vector.tensor_reduce(combo[:, 1:2], goldacc, axis=X_AX,
                                    op=ALU.add)
            res_ps = up.tile([128, 2 * GB], f32, tag="u")
            nc.tensor.matmul(res_ps[0:2, 0:1], lhsT=combo, rhs=ones,
                             start=True, stop=True)
            outsb = singles.tile([2, 1], f32)
            nc.vector.tensor_copy(out=outsb, in_=res_ps[0:2, 0:1])
            nc.sync.dma_start(out=OUTd.ap(), in_=outsb)

    nc.compile()
    return nc


def _get_program():
    if "nc" not in _cache:
        _cache["nc"] = _build_program()
    return _cache["nc"]


def _pack_labels(lab):
    """[4, 8192] int8 label rows -> [32, 1024] packed (1 KB DMA lines) so a
    DMA from rows 0-31 of the label region into [4, 8192] reproduces them."""
    flat = lab.reshape(-1)                       # j = p*8192 + c
    return flat.reshape(32, YW)                  # row p' holds j = p'*1024 ...


def _make_consts(W, T, bd_scale):
    import ml_dtypes
    bf = ml_dtypes.bfloat16
    cb = np.zeros((128, CBW), dtype=np.float64)
    cb[:, :K] = W.astype(bf).astype(np.float64)
    expTs = np.exp(T.astype(np.float64)) / bd_scale
    Tb = T.astype(bf).astype(np.float64)
    for g in range(GROUPS):
        r = slice(32 * g, 32 * g + K)
        cb[r, 32 + 32 * g:32 + 32 * g + K] = Tb       # tb (blockdiag T)
        cb[r, 160 + 32 * g:160 + 32 * g + K] = expTs  # bd (blockdiag expT/s)
        cb[g, 416 + 32 * g:416 + 32 * g + 32] = 1     # egrp (group select)
    for r in range(128):
        g = r // 32
        if r % 32 < K:
            cb[r, 288 + 32 * g:288 + 32 * g + 32] = 1  # zs (group-sum)
        cb[r, 544] = r % 32                            # iota (label index)
    # two-plane int8 decomposition: cb ~= A + B/128
    A = np.clip(np.round(cb), -127, 127)
    Bp = np.clip(np.round((cb - A) * 128.0), -127, 127)
    return A.astype(np.int8), Bp.astype(np.int8)


def _make_in_maps(X, y, W, T):
    import ml_dtypes
    W = np.asarray(W)
    T = np.asarray(T)
    bd_scale = max(1.0, float(np.exp(T.astype(np.float64)).max()) / 120.0)
    _cache["bd_scale"] = bd_scale
    cba, cbb = _make_consts(W, T, bd_scale)

    X = np.asarray(X, dtype=np.float32)
    y = np.asarray(y)
    in_maps = []
    for cidx in range(N_CORES):
        Xc = X[cidx * BC:(cidx + 1) * BC]               # [1024, 32, 128]
        Xg = Xc.reshape(GROUPS, GB, L, F)               # [g, b, t, f]
        # X cols = (tile, group, t_local, b): i*2048 + g*512 + tl*256 + b
        XT = (Xg.transpose(3, 2, 0, 1)                  # [f, t, g, b]
                .reshape(F, NT, 2, GROUPS, GB)          # [f, i, tl, g, b]
                .transpose(0, 1, 3, 2, 4)               # [f, i, g, tl, b]
                .reshape(F, XCOL))
        Xq = np.ascontiguousarray(XT).astype(ml_dtypes.float8_e3m4).view(np.int8)

        Yc = y[cidx * BC:(cidx + 1) * BC].astype(np.int64)  # [1024, 32]
        Yg = Yc.reshape(GROUPS, GB, L)                  # [g, b, t]

        # label rows: [g, (tile, t_local, b)] = [4, i*512 + tl*256 + b]
        def lrows(lbl):
            return (lbl.transpose(0, 2, 1)              # [g, t, b]
                       .reshape(GROUPS, MCOL).astype(np.int8))

        allt = np.zeros((128, WT), dtype=np.int8)
        allt[:, :XCOL] = Xq
        allt[0:32, C_YT:C_YT + YW] = _pack_labels(lrows(Yg))
        allt[:, C_CBA:C_CBA + CBW] = cba
        allt[:, C_CBB:C_CBB + CBW] = cbb
        in_maps.append({"ALL": allt})
    return in_maps


def _combine(results):
    bd_scale = _cache.get("bd_scale", 1.0)
    lncorr = BC * (L - 1) * np.log(bd_scale)
    total = 0.0
    for r in results:
        o = np.asarray(r["out"], dtype=np.float64).reshape(-1)
        sumlog = o[0] / 32.0
        gold = o[1]
        total += gold - (sumlog + lncorr) - BC * L * SHIFT
    return np.float32(total / B)


def kernel(X, y, W, T):
    from concourse.bass_utils import run_bass_kernel_spmd
    nc = _get_program()
    in_maps = _make_in_maps(np.asarray(X), np.asarray(y),
                            np.asarray(W), np.asarray(T))
    res = run_bass_kernel_spmd(nc, in_maps, list(range(N_CORES)))
    return _combine(res.results)


# revision 19
# speedup vs baseline: 1.6074x; 1.1308x over previous
"""CRF loss kernel for Trainium2 (8 NeuronCores, data-parallel over batch).

Reference computation (see problem):
    score = einsum('blf,fk->blk', X, W);  forward/backward CRF messages over L;
    loss = mean_b(emit + trans - logZ).

The per-exec harness cost is dominated by per-tensor staging overhead, so ALL
inputs ship as ONE int8 tensor per core (~4.2 MiB):
  cols [0, 32768)      X as fp8-e3m4 BIT PATTERNS (int8 dtype keeps the fast
                       staging path), X^T in [F=128, (tile, group, t, b)] layout
  cols [32768, 33792)  YT labels y[b,t] packed [4,8192]->[32,1024] (1KB lines)
  cols [33792, 34340)  A-plane of bf16 consts CB (round(c))
  cols [34340, 34888)  B-plane (round((c-A)*128)); device cb = A + B/128
CB = [W pad | blockdiag(T) | blockdiag(expT/s) | group-sum | group-select E |
label iota]; s rescales expT into int8-decomp range and is corrected on host.

Device algorithm (per core, batch shard of 1024):
  - decode consts + label rows; X tiles are DMA'd as raw bytes and the PE
    reads them as fp8 directly via AP bitcast (no decode pass).
  (labels are encoded as k/2, exact in e3m4, so the one-hot equality against
  the halved iota is preserved bit-exactly)
  - 16 pipelined tiles of 512 columns (2 timesteps x 256 batch, 4 batch
    groups packed on partitions at offsets 0/32/64/96):
      score psum = W^T @ XT tile (4 matmuls, tile_position packing)
      expsc = exp(score - SHIFT) via ACT (fused PSUM->SBUF), bf16
      masks on device: yrep = E^T @ YT row (partition-broadcast), then
      oh = is_equal(yrep, iota) on DVE into a per-timestep buffer whose
      one-block-shifted slice doubles as the prev-label mask
      tcol = blockdiag(T)^T @ ohp accumulated INTO the score psum after the
      exp read; gold tile sum = reduce((score + tcol) .* oh)
  - CRF forward recursion in probability domain, interleaved with the tile
    loop: p_t = (BD^T @ p_{t-1}) * expsc_t, renormalized every 3 steps by
    group-sum Z, accumulating log Z via ACT Ln accum_out.
  - out[2,1]: [32*sum_b sum log Z, emit+trans total]
Host: loss = sum_cores(gold - sumlog/32 - BC*L*SHIFT - BC*(L-1)*ln s) / B.
"""

import numpy as np

B, L, F, K = 8192, 32, 128, 26
N_CORES = 8
BC = B // N_CORES            # batch per core
GROUPS = 4                   # batch groups packed on partition blocks
GB = BC // GROUPS            # 256 batch columns per group
NT = L // 2                  # 16 tiles, 2 timesteps each
TILE_COLS = 2 * GB           # 512 columns per tile
SHIFT = 26.0

XCOL = NT * TILE_COLS * GROUPS          # 32768 X-code columns
MCOL = NT * TILE_COLS                   # 8192 mask/expsc columns
YW = 1024                              # label region cols (rows 0-31 used)
CBW = 548                               # const columns
C_YT = XCOL
C_CBA = C_YT + YW
C_CBB = C_CBA + CBW
WT = C_CBB + CBW                        # 34120 total columns

_cache = {}


def _build_program():
    import concourse.bass as bass  # noqa: F401
    import concourse.bacc as bacc
    import concourse.tile as tile
    from concourse import mybir
    from contextlib import ExitStack

    f32 = mybir.dt.float32
    bf16 = mybir.dt.bfloat16
    i8 = mybir.dt.int8
    fp8 = mybir.dt.float8e3
    AF = mybir.ActivationFunctionType
    ALU = mybir.AluOpType
    X_AX = mybir.AxisListType.X

    nc = bacc.Bacc("TRN2", target_bir_lowering=False)

    ALLd = nc.dram_tensor("ALL", [128, WT], i8, kind="ExternalInput")
    OUTd = nc.dram_tensor("out", [2, 1], f32, kind="ExternalOutput")

    with tile.TileContext(nc) as tc, ExitStack() as ctx:
        singles = ctx.enter_context(tc.tile_pool(name="singles", bufs=1))

        # ---- const + label decode ----
        cba8 = singles.tile([128, CBW], i8)
        nc.sync.dma_start(out=cba8, in_=ALLd.ap()[:, C_CBA:C_CBB])
        cbb8 = singles.tile([128, CBW], i8)
        nc.sync.dma_start(out=cbb8, in_=ALLd.ap()[:, C_CBB:C_CBB + CBW])
        cbtmp = singles.tile([128, CBW], bf16)
        nc.vector.tensor_scalar(cbtmp, cbb8, scalar1=1.0 / 128.0, scalar2=None,
                                op0=ALU.mult)
        cba = singles.tile([128, CBW], bf16)
        nc.vector.tensor_copy(out=cba, in_=cba8)
        cb = singles.tile([128, CBW], bf16)
        nc.vector.tensor_tensor(cb, cba, cbtmp, ALU.add)

        wblk = cb[:, 0:32]
        tb = cb[:, 32:160]
        bd = cb[:, 160:288]
        zs = cb[:, 288:416]
        egrp = cb[0:GROUPS, 416:544]        # E[g, 32g'+k] = (g == g')
        iota = cb[:, 544:545]               # partition index % 32

        yt8 = singles.tile([GROUPS, MCOL], i8)
        nc.scalar.dma_start(out=yt8, in_=ALLd.ap()[0:32, C_YT:C_YT + YW])
        yt = yt8.bitcast(fp8)           # labels encoded as k/2 (exact in e3m4)
        # one-hot per timestep: block j holds onehot(y at t=j-1); block 0 = 0
        ohbuf = singles.tile([128, (L + 1) * GB], bf16)
        nc.vector.memset(ohbuf[:, 0:GB], 0.0)

        expsc = singles.tile([128, MCOL], bf16)

        nshift = singles.tile([128, 1], f32)
        nc.vector.memset(nshift, -SHIFT)
        ones = singles.tile([128, 1], f32)
        nc.vector.memset(ones, 1.0)
        goldacc = singles.tile([128, NT], f32)
        nc.vector.memset(goldacc, 0.0)
        logacc = singles.tile([128, 16], f32)
        nc.vector.memset(logacc, 0.0)
        combo = singles.tile([128, 2], f32)

        with tc.tile_pool(name="x8", bufs=3) as x8p, \
             tc.tile_pool(name="scp", bufs=2, space="PSUM") as scp, \
             tc.tile_pool(name="yrp", bufs=2, space="PSUM") as yrp, \
             tc.tile_pool(name="mp", bufs=2) as mp, \
             tc.tile_pool(name="up", bufs=2, space="PSUM") as up, \
             tc.tile_pool(name="vp", bufs=2) as vp, \
             tc.tile_pool(name="rzp", bufs=2) as rzp, \
             tc.tile_pool(name="lnp", bufs=2) as lnp, \
             tc.tile_pool(name="pp", bufs=2) as pp:

            p_prev = None
            nidx = 0

            def recursion_step(t):
                nonlocal p_prev, nidx
                ctx2 = tc.high_priority()
                ctx2.__enter__()
                u = up.tile([128, 2 * GB], f32, tag="u")
                nc.tensor.matmul(u[:, 0:GB], lhsT=bd, rhs=p_prev,
                                 start=True, stop=True)
                i = t // 2
                e_sl = expsc[:, i * TILE_COLS + (t % 2) * GB:
                             i * TILE_COLS + (t % 2) * GB + GB]
                if t % 3 == 0:
                    v = vp.tile([128, GB], bf16)
                    nc.vector.tensor_mul(v, u[:, 0:GB], e_sl)
                    z = u[:, GB:2 * GB]
                    nc.tensor.matmul(z, lhsT=zs, rhs=v, start=True, stop=True,
                                     skip_group_check=True)
                    rz = rzp.tile([128, GB], f32)
                    nc.vector.reciprocal(rz, z)
                    lnscr = lnp.tile([128, GB], bf16)
                    nc.scalar.activation(lnscr, z, AF.Ln,
                                         accum_out=logacc[:, nidx:nidx + 1])
                    nidx += 1
                    pn = pp.tile([128, GB], bf16)
                    nc.vector.tensor_mul(pn, v, rz)
                else:
                    pn = pp.tile([128, GB], bf16)
                    nc.vector.tensor_mul(pn, u[:, 0:GB], e_sl)
                ctx2.__exit__(None, None, None)
                p_prev = pn

            for i in range(NT):
                cs = slice(i * TILE_COLS, (i + 1) * TILE_COLS)

                x8i = x8p.tile([128, GROUPS * TILE_COLS], i8)
                nc.sync.dma_start(
                    out=x8i,
                    in_=ALLd.ap()[:, i * GROUPS * TILE_COLS:
                                  (i + 1) * GROUPS * TILE_COLS])
                xti = x8i.bitcast(fp8)

                # one-hot masks: write blocks for t=2i, 2i+1; the prev-label
                # mask is the same buffer shifted one timestep back
                yrep = yrp.tile([128, TILE_COLS], f32, tag="yr")
                nc.tensor.matmul(yrep, lhsT=egrp, rhs=yt[:, cs],
                                 start=True, stop=True)
                nc.vector.tensor_tensor(
                    ohbuf[:, (2 * i + 1) * GB:(2 * i + 3) * GB], yrep,
                    iota.to_broadcast([128, TILE_COLS]), ALU.is_equal)
                oh_t = ohbuf[:, (2 * i + 1) * GB:(2 * i + 3) * GB]
                ohp_t = ohbuf[:, (2 * i) * GB:(2 * i + 2) * GB]

                sc = scp.tile([128, TILE_COLS], f32)
                for g in range(GROUPS):
                    nc.tensor.matmul(
                        sc[32 * g:32 * g + 32, :],
                        lhsT=wblk,
                        rhs=xti[:, g * TILE_COLS:(g + 1) * TILE_COLS],
                        start=True, stop=True,
                        tile_position=(0, 32 * g),
                    )
                nc.scalar.activation(expsc[:, cs], sc, AF.Exp,
                                     bias=nshift[:, 0:1])

                # accumulate T[y_prev, k] into the score psum (after exp read)
                nc.tensor.matmul(sc, lhsT=tb, rhs=ohp_t,
                                 start=False, stop=True,
                                 skip_group_check=True)

                m1 = mp.tile([128, TILE_COLS], f32)
                nc.vector.tensor_tensor(m1, sc, oh_t, ALU.mult)
                mscr = mp.tile([128, TILE_COLS], f32, tag="mscr")
                nc.scalar.activation(mscr, m1, AF.Copy,
                                     accum_out=goldacc[:, i:i + 1])

                # recursion steps enabled by this tile
                if i == 0:
                    p_prev = expsc[:, 0:GB]
                    recursion_step(1)
                else:
                    recursion_step(2 * i)
                    recursion_step(2 * i + 1)

            # final partition-function sum (t = L-1 state)
            zf = up.tile([128, 2 * GB], f32, tag="u")
            nc.tensor.matmul(zf[:, 0:GB], lhsT=zs, rhs=p_prev,
                             start=True, stop=True)
            lnscr = lnp.tile([128, GB], bf16)
            nc.scalar.activation(lnscr, zf[:, 0:GB], AF.Ln,
                                 accum_out=logacc[:, nidx:nidx + 1])
            nidx += 1
            assert nidx == 11

            nc.vector.tensor_reduce(combo[:, 0:1], logacc, axis=X_AX,
                                    op=ALU.add)
            nc.vector.tensor_reduce(combo[:, 1:2], goldacc, axis=X_AX,
                                    op=ALU.add)
            res_ps = up.tile([128, 2 * GB], f32, tag="u")
            nc.tensor.matmul(res_ps[0:2, 0:1], lhsT=combo, rhs=ones,
                             start=True, stop=True)
            outsb = singles.tile([2, 1], f32)
            nc.vector.tensor_copy(out=outsb, in_=res_ps[0:2, 0:1])
            nc.sync.dma_start(out=OUTd.ap(), in_=outsb)

    nc.compile()
    return nc


def _get_program():
    if "nc" not in _cache:
        _cache["nc"] = _build_program()
    return _cache["nc"]


def _pack_labels(lab):
    """[4, 8192] int8 label rows -> [32, 1024] packed (1 KB DMA lines) so a
    DMA from rows 0-31 of the label region into [4, 8192] reproduces them."""
    flat = lab.reshape(-1)                       # j = p*8192 + c
    return flat.reshape(32, YW)                  # row p' holds j = p'*1024 ...


def _make_consts(W, T, bd_scale):
    import ml_dtypes
    bf = ml_dtypes.bfloat16
    cb = np.zeros((128, CBW), dtype=np.float64)
    cb[:, :K] = W.astype(bf).astype(np.float64)
    expTs = np.exp(T.astype(np.float64)) / bd_scale
    Tb = T.astype(bf).astype(np.float64)
    for g in range(GROUPS):
        r = slice(32 * g, 32 * g + K)
        cb[r, 32 + 32 * g:32 + 32 * g + K] = Tb       # tb (blockdiag T)
        cb[r, 160 + 32 * g:160 + 32 * g + K] = expTs  # bd (blockdiag expT/s)
        cb[g, 416 + 32 * g:416 + 32 * g + 32] = 1     # egrp (group select)
    for r in range(128):
        g = r // 32
        if r % 32 < K:
            cb[r, 288 + 32 * g:288 + 32 * g + 32] = 1  # zs (group-sum)
        cb[r, 544] = (r % 32) / 2.0                    # iota (label idx / 2)
    # two-plane int8 decomposition: cb ~= A + B/128
    A = np.clip(np.round(cb), -127, 127)
    Bp = np.clip(np.round((cb - A) * 128.0), -127, 127)
    return A.astype(np.int8), Bp.astype(np.int8)


def _make_in_maps(X, y, W, T):
    import ml_dtypes
    W = np.asarray(W)
    T = np.asarray(T)
    bd_scale = max(1.0, float(np.exp(T.astype(np.float64)).max()) / 120.0)
    _cache["bd_scale"] = bd_scale
    cba, cbb = _make_consts(W, T, bd_scale)

    X = np.asarray(X, dtype=np.float32)
    y = np.asarray(y)
    in_maps = []
    for cidx in range(N_CORES):
        Xc = X[cidx * BC:(cidx + 1) * BC]               # [1024, 32, 128]
        Xg = Xc.reshape(GROUPS, GB, L, F)               # [g, b, t, f]
        # X cols = (tile, group, t_local, b): i*2048 + g*512 + tl*256 + b
        XT = (Xg.transpose(3, 2, 0, 1)                  # [f, t, g, b]
                .reshape(F, NT, 2, GROUPS, GB)          # [f, i, tl, g, b]
                .transpose(0, 1, 3, 2, 4)               # [f, i, g, tl, b]
                .reshape(F, XCOL))
        Xq = np.ascontiguousarray(XT).astype(ml_dtypes.float8_e3m4).view(np.int8)

        Yc = y[cidx * BC:(cidx + 1) * BC].astype(np.int64)  # [1024, 32]
        Yg = Yc.reshape(GROUPS, GB, L)                  # [g, b, t]

        # label rows as fp8(k/2) bytes: [g, (tile, t_local, b)]
        def lrows(lbl):
            lab = (lbl.transpose(0, 2, 1)               # [g, t, b]
                      .reshape(GROUPS, MCOL).astype(np.float64) / 2.0)
            return np.ascontiguousarray(
                lab.astype(ml_dtypes.float8_e3m4)).view(np.int8)

        allt = np.zeros((128, WT), dtype=np.int8)
        allt[:, :XCOL] = Xq
        allt[0:32, C_YT:C_YT + YW] = _pack_labels(lrows(Yg))
        allt[:, C_CBA:C_CBA + CBW] = cba
        allt[:, C_CBB:C_CBB + CBW] = cbb
        in_maps.append({"ALL": allt})
    return in_maps


def _combine(results):
    bd_scale = _cache.get("bd_scale", 1.0)
    lncorr = BC * (L - 1) * np.log(bd_scale)
    total = 0.0
    for r in results:
        o = np.asarray(r["out"], dtype=np.float64).reshape(-1)
        sumlog = o[0] / 32.0
        gold = o[1]
        total += gold - (sumlog + lncorr) - BC * L * SHIFT
    return np.float32(total / B)


def kernel(X, y, W, T):
    from concourse.bass_utils import run_bass_kernel_spmd
    nc = _get_program()
    in_maps = _make_in_maps(np.asarray(X), np.asarray(y),
                            np.asarray(W), np.asarray(T))
    res = run_bass_kernel_spmd(nc, in_maps, list(range(N_CORES)))
    return _combine(res.results)
